# revision 47
# baseline (speedup 1.0000x reference)
"""Causal (prefix) attention block for Trainium2, 8 NeuronCores.

Problem: x:[2,2048,1024] -> qkv = x@w_qkv -> 16-head causal attention
         (the "prefix" part of the reference mask is provably a no-op:
         it clears cells that are already below the diagonal) -> @w_proj.

Sharding: core c = 4*b + g  (b: batch 0..1, g: head-group 0..3, 4 heads each).
Each core computes its head-group's attention output and a PARTIAL
projection y_c = attn_out_g @ w_proj[rows of g]; host sums the 4 partials
per batch (the data-parallel gather-reduce).

Per-core layout trick: everything is computed transposed-by-blocks so the
contraction dim always sits on SBUF partitions:
  qT,kT: [head-dims(128=2 heads), L]   (lhsT = w chunk, rhs = xT chunk)
  v_aug: [L-chunk(128), 4*(64+1)]      per head: [v | ones] -> AV matmul
                                        yields [out^T ; softmax-denominator]
  attention: two logit tiles pair into one [128,1024] PSUM tile -> ONE exp
             per pair (amortizes ACT per-op overhead; no max-subtraction:
             logits ~ N(0,1), exp is safe in fp32); causal mask = DVE multiply
             with sliding slices of one precomputed [128,1024] 0/1 mask;
             AV accumulates over lk in PSUM; softmax denominators divide out
             via vector.reciprocal + gpsimd.partition_broadcast.
  proj: lhsT = attn^T chunk (native output of AV), rhs = w_proj rows chunk,
        interleaved between attention head groups of the next lq tile so the
        PE stays fed while ACT's exp stream catches up.

All matmuls use float32r (TF32-ish: full-rate on PE for moving dim >= 256,
fp32 accumulate in PSUM).  All PSUM pools coexist in exactly 8 banks (psl
pairs 2x2 + five 1-bank rotating slots) so the three phases pipeline on the
PE with no pool-recycling barriers; QKV runs as two contraction passes so
matmuls start while x is still loading.
"""

import time

import numpy as np

P = 128
B, L, D = 2, 2048, 1024
H, DH = 16, 64
HPC = 4               # heads per core
GC = HPC * DH         # 256 per-core head dims
VW = DH + 1           # v columns per head incl. ones column
LQT, LKT = 512, 128
NLQ, NLK = L // LQT, L // LKT
NDC = D // P          # 8 contraction chunks
NCORES = 8

_CACHE: dict = {}


def _build_nc():
    from contextlib import ExitStack

    import concourse.mybir as mybir
    import concourse.tile as tile
    from concourse import bacc

    dtr = mybir.dt.float32r
    f32 = mybir.dt.float32
    Exp = mybir.ActivationFunctionType.Exp
    mult = mybir.AluOpType.mult

    nc = bacc.Bacc("TRN2", target_bir_lowering=False, debug=False,
                   num_devices=NCORES)
    xT = nc.dram_tensor("xT", [D, L], dtr, kind="ExternalInput").ap()
    wq = nc.dram_tensor("wq", [D, GC], dtr, kind="ExternalInput").ap()
    wk = nc.dram_tensor("wk", [D, GC], dtr, kind="ExternalInput").ap()
    wv = nc.dram_tensor("wv", [D, GC], dtr, kind="ExternalInput").ap()
    wp = nc.dram_tensor("wp", [GC, D], dtr, kind="ExternalInput").ap()
    msk = nc.dram_tensor("msk", [P, 2 * LQT], dtr, kind="ExternalInput").ap()
    y = nc.dram_tensor("y", [L, D], f32, kind="ExternalOutput").ap()

    with ExitStack() as ctx:
        tc = ctx.enter_context(tile.TileContext(nc))
        per = ctx.enter_context(tc.tile_pool(name="persist", bufs=1))
        xw = ctx.enter_context(tc.tile_pool(name="xw", bufs=1))
        esb = ctx.enter_context(tc.tile_pool(name="esb", bufs=5))
        nrm = ctx.enter_context(tc.tile_pool(name="nrm", bufs=2))
        ysbp = ctx.enter_context(tc.tile_pool(name="ysb", bufs=4))
        # PSUM: psl pairs get 2 dedicated 2-bank slots; pq/pk/pv/pso/psy
        # share 4 one-bank slots = 8 banks exactly
        pbig = ctx.enter_context(tc.tile_pool(name="pbig", bufs=2,
                                              space="PSUM"))
        pvy = ctx.enter_context(tc.tile_pool(name="pvy", bufs=4, space="PSUM"))
        psop = pvy

        SPLIT = 4
        qT = [per.tile([P, L], dtr, tag=f"qT{i}", name=f"qT{i}")
              for i in range(2)]
        kT = [per.tile([P, L], dtr, tag=f"kT{i}", name=f"kT{i}")
              for i in range(2)]
        vA = [per.tile([P, HPC * VW], dtr, tag=f"v{i}", name=f"v{i}")
              for i in range(NLK)]
        aT = [per.tile([P, L], dtr, tag=f"aT{i}", name=f"aT{i}")
              for i in range(2)]

        # ------- input DMAs: x bulk on SP-HWDGE, weights on ACT-HWDGE ----
        xTt = []
        for i in range(NDC):
            t = xw.tile([P, L], dtr, tag=f"x{i}", name=f"x{i}")
            if i < 1:
                # split pass-A chunks so the in-order QKV matmul stream
                # (which reads 512-col slices) can progress per half-chunk
                r = slice(i * P, (i + 1) * P)
                nc.sync.dma_start(t[:, 0:L // 2], xT[r, 0:L // 2])
                nc.sync.dma_start(t[:, L // 2:L], xT[r, L // 2:L])
            else:
                nc.sync.dma_start(t[:], xT[i * P:(i + 1) * P, :])
            xTt.append(t)
        wts = {}
        for nm, w in (("wq", wq), ("wk", wk), ("wv", wv)):
            wts[nm] = []
            for i in range(NDC):
                t = xw.tile([P, GC], dtr, tag=f"{nm}{i}", name=f"{nm}{i}")
                nc.scalar.dma_start(t[:], w[i * P:(i + 1) * P, :])
                wts[nm].append(t)
        wpt = []
        for i in range(2):
            t = xw.tile([P, D], dtr, tag=f"wp{i}", name=f"wp{i}")
            nc.scalar.dma_start(t[:], wp[i * P:(i + 1) * P, :])
            wpt.append(t)
        mk = per.tile([P, 2 * LQT], dtr, tag="mask", name="mask")
        nc.scalar.dma_start(mk[:], msk)

        # ---------------- QKV projection ----------------
        # two contraction passes: pass A (dc 0-3) can start once the first
        # half of x has arrived; pass B (dc 4-7) adds on top via DVE.
        add = mybir.AluOpType.add
        for half in range(2):
            dcs = range(0, SPLIT) if half == 0 else range(SPLIT, NDC)
            for ch in range(2):
                cs = slice(ch * P, (ch + 1) * P)
                for lq in range(NLQ):
                    qs = slice(lq * LQT, (lq + 1) * LQT)
                    pq = pvy.tile([P, LQT], f32, tag="vy", name="pq")[:]
                    pk = pvy.tile([P, LQT], f32, tag="vy", name="pk")[:]
                    nd = len(dcs)
                    for i, dc in enumerate(dcs):
                        nc.tensor.matmul(pq, wts["wq"][dc][:, cs],
                                         xTt[dc][:, qs],
                                         start=i == 0, stop=i == nd - 1)
                    for i, dc in enumerate(dcs):
                        nc.tensor.matmul(pk, wts["wk"][dc][:, cs],
                                         xTt[dc][:, qs],
                                         start=i == 0, stop=i == nd - 1)
                    if half == 0:
                        nc.vector.tensor_copy(qT[ch][:, qs], pq)
                        nc.vector.tensor_copy(kT[ch][:, qs], pk)
                    else:
                        nc.vector.tensor_tensor(qT[ch][:, qs], pq,
                                                qT[ch][:, qs], add)
                        nc.vector.tensor_tensor(kT[ch][:, qs], pk,
                                                kT[ch][:, qs], add)
            for lc in range(NLK):
                ls = slice(lc * P, (lc + 1) * P)
                pv = pvy.tile([P, LQT], f32, tag="vy", name="pv")
                nd = len(dcs)
                for i, dc in enumerate(dcs):
                    nc.tensor.matmul(pv[:, 0:GC], xTt[dc][:, ls],
                                     wts["wv"][dc][:],
                                     start=i == 0, stop=i == nd - 1)
                va = vA[lc]
                v3 = va[:, 0:HPC * VW].rearrange("p (h x) -> p h x", h=HPC)
                pv3 = pv[:, 0:GC].rearrange("p (h d) -> p h d", h=HPC)
                if half == 0:
                    nc.vector.memset(v3[:, :, DH:VW].bitcast(f32), 1.0)
                    nc.vector.tensor_copy(v3[:, :, 0:DH], pv3)
                else:
                    nc.vector.tensor_tensor(v3[:, :, 0:DH], pv3,
                                            v3[:, :, 0:DH], add)

        # ---------------- attention (lq outer) + interleaved projection ---
        scale = float(DH) ** -0.5

        def proj_half(lc, nn):
            ls = slice(lc * P, (lc + 1) * P)
            ns = slice(nn * 512, (nn + 1) * 512)
            psy = pvy.tile([P, LQT], f32, tag="vy", name="psy")
            for kc in range(2):
                nc.tensor.matmul(psy[:], aT[kc][:, ls], wpt[kc][:, ns],
                                 start=kc == 0, stop=kc == 1)
            ysb = ysbp.tile([P, LQT], f32, tag="ysb", name="ysb")
            nc.vector.tensor_copy(ysb[:], psy[:])
            nc.sync.dma_start(y[ls, ns], ysb[:])

        def proj_lc(lc):
            proj_half(lc, 0)
            proj_half(lc, 1)

        for lq in range(NLQ):
            q0 = lq * LQT
            qs = slice(q0, q0 + LQT)
            nlk = q0 // LKT + LQT // LKT
            horder = (1, 3, 0, 2) if lq == NLQ - 1 else (0, 1, 2, 3)
            for h in horder:
                ch, off = h // 2, (h % 2) * DH
                hs = slice(off, off + DH)
                pso_t = psop.tile([P, LQT], f32, tag="vy", name="pso")
                pso = pso_t[0:VW, :]
                for pr in range(nlk // 2):
                    lk0, lk1 = 2 * pr, 2 * pr + 1
                    psl = pbig.tile([P, 2 * LQT], f32, tag="big", name="psl")
                    e = esb.tile([P, 2 * LQT], dtr, tag="e", name="e")
                    # a diagonal tile at key-offset r only affects queries
                    # f >= r: stream only the last w columns (clamped to 256
                    # so float32r stays full-rate), packed contiguously so a
                    # single narrower exp covers both halves.
                    halves, off0 = [], 0
                    for lk in (lk0, lk1):
                        r = max(lk * LKT - q0, 0)
                        w = min(LQT, max(256, LQT - r))
                        halves.append((lk, r, w, off0))
                        off0 += w
                    for lk, r, w, o in halves:
                        nc.tensor.matmul(
                            psl[:, o:o + w],
                            kT[ch][hs, lk * LKT:(lk + 1) * LKT],
                            qT[ch][hs, q0 + LQT - w:q0 + LQT],
                            start=True, stop=True)
                    nc.scalar.activation(e[:, 0:off0], psl[:, 0:off0],
                                         Exp, scale=scale)
                    for lk, r, w, o in halves:
                        if lk * LKT - q0 < 0:
                            continue
                        rr = r - LQT + w   # residual triangle offset in-range
                        nc.vector.tensor_tensor(
                            e[:, o:o + w], e[:, o:o + w],
                            mk[:, LQT - rr:LQT - rr + w], mult)
                    for lk, r, w, o in halves:
                        nc.tensor.matmul(pso[0:VW, LQT - w:LQT],
                                         vA[lk][:, h * VW:(h + 1) * VW],
                                         e[:, o:o + w], start=lk == 0,
                                         stop=lk == nlk - 1)
                # interleave projection of the previous lq region between
                # attention head groups to fill PE while ACT catches up
                if lq >= 1:
                    proj_lc(4 * (lq - 1) + h)
                rec = nrm.tile([1, LQT], f32, tag="rec", name="rec")
                nc.vector.reciprocal(rec[:], pso[DH:VW, :])
                bc = nrm.tile([DH, LQT], f32, tag="bc", name="bc")
                nc.gpsimd.partition_broadcast(bc[:], rec[:])
                if off == 0:
                    nc.vector.tensor_tensor(aT[ch][hs, qs], pso[0:DH, :],
                                            bc[:], mult)
                else:
                    tmp = nrm.tile([DH, LQT], dtr, tag="tmp", name="tmp")
                    nc.vector.tensor_tensor(tmp[:], pso[0:DH, :], bc[:], mult)
                    nc.sync.dma_start(aT[ch][hs, qs], tmp[:])
        for lc in range(4 * (NLQ - 1), NLK):
            proj_lc(lc)

    nc.compile()
    return nc


def _mask_big() -> np.ndarray:
    # mk[p, c] = 1.0 where (c - LQT) >= p else 0; a narrowed diagonal region
    # with residual triangle offset rr uses the slice starting at LQT - rr.
    p = np.arange(P)[:, None]
    c = np.arange(2 * LQT)[None, :]
    return ((c - LQT) >= p).astype(np.float32)


def kernel(x, w_qkv, w_proj, num_prefix_tokens=None):
    """Full inputs in, full output out. num_prefix_tokens is a provable
    no-op in the reference mask (it only clears already-clear cells)."""
    from concourse.bass_utils import run_bass_kernel_spmd

    x = np.asarray(x, dtype=np.float32)
    w_qkv = np.asarray(w_qkv, dtype=np.float32)
    w_proj = np.asarray(w_proj, dtype=np.float32)

    if "nc" not in _CACHE:
        _CACHE["nc"] = _build_nc()
    nc = _CACHE["nc"]

    mask = _mask_big()
    xTs = [np.ascontiguousarray(x[b].T) for b in range(B)]
    in_maps = []
    for c in range(NCORES):
        b, g = divmod(c, HPC)
        gs = slice(GC * g, GC * (g + 1))
        in_maps.append({
            "xT": xTs[b],
            "wq": np.ascontiguousarray(w_qkv[:, 0 * D:1 * D][:, gs]),
            "wk": np.ascontiguousarray(w_qkv[:, 1 * D:2 * D][:, gs]),
            "wv": np.ascontiguousarray(w_qkv[:, 2 * D:3 * D][:, gs]),
            "wp": np.ascontiguousarray(w_proj[gs, :]),
            "msk": mask,
        })

    t0 = time.perf_counter()
    res = run_bass_kernel_spmd(nc, in_maps, core_ids=list(range(NCORES)))
    _CACHE["last_run_s"] = time.perf_counter() - t0

    out = np.empty((B, L, D), dtype=np.float32)
    for b in range(B):
        acc = res.results[HPC * b]["y"].copy()
        for g in range(1, HPC):
            acc += res.results[HPC * b + g]["y"]
        out[b] = acc
    return out
